# revision 2
# baseline (speedup 1.0000x reference)
"""Pairwise cosine similarity [8192, 8192] on 8 Trainium2 NeuronCores.

out[n, m] = dot(input1[n], input2[m]) / max(||input1[n]|| * ||input2[m]||, eps)

Sharding: rows of input1 (N) are split across the 8 cores; input2 is
replicated. Each core computes a [1024, 8192] slab of the output.

Device kernel (per core), D = 512 contraction dim, all fp16 operands:
  - Inputs arrive host-transposed as x1t [512, 1024] / x2t [512, 8192]
    (d-major) so the TensorE contraction needs no on-chip transposes.
  - Both operands are PRE-SCALED on device by their inverse norms
    (x1s = x1/||x1||, x2s = x2/||x2||), so the main matmul accumulates the
    final cosine directly in PSUM and the PSUM drain is a pure fp32->fp16
    copy.  This replaces the per-output scalar_tensor_tensor epilogue
    (fp32-PSUM STT runs at 1x on DVE = ~750ns/512-chunk = 96us serial on
    one engine) with per-input fp16 muls (2x mode) + copies that split
    across BOTH ACT and DVE (~570/658ns per chunk).
  - Norms: squares (split ACT/DVE), k-tiles folded pairwise on DVE (3 adds)
    so the ones-stationary partition-reduce matmul is a single pass
    (9216 PE cycles total instead of 36864), then DVE
    reciprocal_approx_fast + ACT sqrt -> fp16 inverse norms replicated
    across partitions.
  - 12 dummy warm-up matmuls run during the load prologue to flip the PE
    HAM clock gate to 8/8 before the real matmuls start (saves ~7us of
    half-rate execution).
  - Column blocks of [1024, 2048, 2048, 2048, 1024]: the small first block
    shortens the load->square->fold->reduce->rsqrt->prescale critical path
    in front of the first main matmul; the small last block shortens the
    drain/store tail.  Norm chains for block b+1 are emitted in the middle
    of block b's mains so every engine queue has them ready at the block
    boundary.
  - Output is stored as fp16 (halves HBM store traffic: DMA total ~25MiB
    = ~70us, under the 109us PE floor); the host upcasts to fp32.
"""

import sys

import numpy as np

sys.path.insert(0, "/opt/trn_rl_repo")

import concourse.bass as bass  # noqa: E402
import concourse.mybir as mybir  # noqa: E402
from concourse import bacc  # noqa: E402
from concourse.tile import TileContext  # noqa: E402
from concourse.bass_utils import run_bass_kernel_spmd  # noqa: E402

N_CORES = 8
N = 8192  # rows of input1 (output rows)
M = 8192  # rows of input2 (output cols)
D = 512  # feature dim (contraction)
N_SHARD = N // N_CORES  # 1024 rows per core

P = 128  # partitions
CHUNK = 512  # matmul free-dim chunk (= fp32 PSUM bank free size)
KT = D // P  # 4 k-tiles
M_TILES = N_SHARD // P  # 8 output row tiles per core

# Column blocks: small first block to shorten the norm-chain prologue in
# front of the first main matmul; small last block to shorten the tail.
BLOCKS = [(0, 1024), (1024, 2048), (3072, 2048), (5120, 2048), (7168, 1024)]
N_WARMUP_MM = 12

DT = mybir.dt.float16
NP_DT = np.float16
F32 = mybir.dt.float32

_CACHE = {}


def _build():
    nc = bacc.Bacc("TRN2", target_bir_lowering=False, debug=False)

    x1t = nc.dram_tensor("x1t", [D, N_SHARD], DT, kind="ExternalInput")
    x2t = nc.dram_tensor("x2t", [D, M], DT, kind="ExternalInput")
    out_d = nc.dram_tensor("out", [N_SHARD, M], DT, kind="ExternalOutput")

    with TileContext(nc) as tc:
        with (
            tc.tile_pool(name="consts", bufs=2) as consts,
            tc.tile_pool(name="x1raw", bufs=KT) as x1raw_pool,
            tc.tile_pool(name="x1s", bufs=KT) as x1s_pool,
            tc.tile_pool(name="x2raw", bufs=len(BLOCKS)) as x2raw_pool,
            tc.tile_pool(name="sq", bufs=6) as sq_pool,
            tc.tile_pool(name="fold", bufs=4) as fold_pool,
            tc.tile_pool(name="rc", bufs=2) as rc_pool,
            tc.tile_pool(name="inv2", bufs=3) as inv2_pool,
            tc.tile_pool(name="stag", bufs=6) as stag_pool,
            tc.tile_pool(name="pnorm", bufs=2, space="PSUM") as pnorm_pool,
            tc.tile_pool(name="pmain", bufs=6, space="PSUM") as pmain_pool,
        ):
            ones = consts.tile([P, P], DT)
            nc.vector.memset(ones[:], 1.0)
            warm_mov = consts.tile([P, CHUNK], DT, tag="warm")
            nc.vector.memset(warm_mov[:], 0.0)

            # ---------- HAM warm-up: dummy matmuls with no DMA deps ----
            for _ in range(N_WARMUP_MM):
                wp = pmain_pool.tile([P, CHUNK], F32, tag="pmain")
                nc.tensor.matmul(wp[:], ones[:], warm_mov[:], start=True, stop=True)

            # ---------- loads ----------
            x1t_v = x1t.rearrange("(k p) n -> p k n", p=P)  # [128, 4, 1024]
            x2t_v = x2t.rearrange("(k p) m -> p k m", p=P)  # [128, 4, 8192]

            # Block 0 per-k loads first (its norm chain is the kernel's
            # critical-path prologue), then x1 per-k, then fused blocks.
            b0_off, b0_w = BLOCKS[0]
            x2big = {}
            t0 = x2raw_pool.tile([P, KT * b0_w], DT, tag="x2raw")
            for k in range(KT):
                nc.sync.dma_start(
                    out=t0[:, k * b0_w : (k + 1) * b0_w],
                    in_=x2t_v[:, k, b0_off : b0_off + b0_w],
                )
            x2big[0] = t0
            x1raw = []
            for k in range(KT):
                t = x1raw_pool.tile([P, N_SHARD], DT, tag="x1raw")
                nc.sync.dma_start(out=t[:], in_=x1t_v[:, k, :])
                x1raw.append(t)
            for b in range(1, len(BLOCKS)):
                off, w = BLOCKS[b]
                t = x2raw_pool.tile([P, KT * w], DT, tag="x2raw")
                nc.sync.dma_start(
                    out=t[:].rearrange("p (k m) -> p k m", k=KT),
                    in_=x2t_v[:, :, off : off + w],
                )
                x2big[b] = t

            def x2sl(b, k, lo, hi):
                w = BLOCKS[b][1]
                return x2big[b][:, k * w + lo : k * w + hi]

            # ---------- x1 chain: norms -> inv1 (replicated) -> x1s ----
            sq1 = []
            for k in range(KT):
                t = fold_pool.tile([P, N_SHARD], DT, tag="x1sq")
                if k % 2 == 0:
                    nc.scalar.square(t[:], x1raw[k][:])
                else:
                    nc.vector.tensor_mul(t[:], x1raw[k][:], x1raw[k][:])
                sq1.append(t)
            f01 = fold_pool.tile([P, N_SHARD], DT, tag="x1f")
            nc.vector.tensor_add(f01[:], sq1[0][:], sq1[1][:])
            f23 = fold_pool.tile([P, N_SHARD], DT, tag="x1f")
            nc.vector.tensor_add(f23[:], sq1[2][:], sq1[3][:])
            s1 = fold_pool.tile([P, N_SHARD], DT, tag="x1f")
            nc.vector.tensor_add(s1[:], f01[:], f23[:])

            rc1 = rc_pool.tile([P, N_SHARD], F32, tag="rc")
            for h in range(N_SHARD // CHUNK):
                hs = slice(h * CHUNK, (h + 1) * CHUNK)
                pn = pnorm_pool.tile([P, CHUNK], F32, tag="pnorm")
                nc.tensor.matmul(pn[:], ones[:], s1[:, hs], start=True, stop=True)
                nc.vector.reciprocal_approx_fast(rc1[:, hs], pn[:])
            inv1rep = consts.tile([P, N_SHARD], DT, tag="inv1")
            nc.scalar.sqrt(inv1rep[:], rc1[:])
            x1s = []
            for k in range(KT):
                t = x1s_pool.tile([P, N_SHARD], DT, tag="x1s")
                nc.vector.tensor_mul(t[:], x1raw[k][:], inv1rep[:])
                x1s.append(t)

            # ---------- x2 norm chain for one block ------------------
            def x2_chain(b):
                off, w = BLOCKS[b]
                sq = []
                for k in range(KT):
                    t = sq_pool.tile([P, w], DT, tag="sq")
                    if k % 2 == 0:
                        nc.scalar.square(t[:], x2sl(b, k, 0, w))
                    else:
                        nc.vector.tensor_mul(t[:], x2sl(b, k, 0, w), x2sl(b, k, 0, w))
                    sq.append(t)
                f01 = fold_pool.tile([P, w], DT, tag="fold")
                nc.vector.tensor_add(f01[:], sq[0][:], sq[1][:])
                f23 = fold_pool.tile([P, w], DT, tag="fold")
                nc.vector.tensor_add(f23[:], sq[2][:], sq[3][:])
                s = fold_pool.tile([P, w], DT, tag="fold")
                nc.vector.tensor_add(s[:], f01[:], f23[:])

                inv2 = inv2_pool.tile([P, w], DT, tag="inv2")
                for h in range(w // N_SHARD):  # 1024-wide rsqrt groups
                    rc = rc_pool.tile([P, N_SHARD], F32, tag="rc")
                    for c in range(N_SHARD // CHUNK):
                        lo = h * N_SHARD + c * CHUNK
                        pn = pnorm_pool.tile([P, CHUNK], F32, tag="pnorm")
                        nc.tensor.matmul(
                            pn[:], ones[:], s[:, lo : lo + CHUNK], start=True, stop=True
                        )
                        nc.vector.reciprocal_approx_fast(
                            rc[:, c * CHUNK : (c + 1) * CHUNK], pn[:]
                        )
                    nc.scalar.sqrt(
                        inv2[:, h * N_SHARD : (h + 1) * N_SHARD], rc[:]
                    )
                # prescale (in place): x2s = x2raw * inv2rep.  Fine-grained
                # (512) on block 0 so the first main matmul starts earlier.
                step = CHUNK if b == 0 else N_SHARD
                for h in range(w // step):
                    lo, hi = h * step, (h + 1) * step
                    for k in range(KT):
                        nc.vector.tensor_mul(
                            x2sl(b, k, lo, hi), x2sl(b, k, lo, hi), inv2[:, lo:hi]
                        )

            # ---------- mains for one block (+ next block's chain) ----
            drain_ct = [0]

            def mains(b):
                off, w = BLOCKS[b]
                n_chunks = w // CHUNK
                for m in range(M_TILES):
                    if m == 4 and b + 1 < len(BLOCKS):
                        x2_chain(b + 1)  # keep the next chain off the
                        # block boundary's critical path
                    stag = stag_pool.tile([P, w], DT, tag="stag")
                    for ci in range(n_chunks):
                        cs = slice(ci * CHUNK, (ci + 1) * CHUNK)
                        ps = pmain_pool.tile([P, CHUNK], F32, tag="pmain")
                        for k in range(KT):
                            nc.tensor.matmul(
                                ps[:],
                                x1s[k][:, m * P : (m + 1) * P],
                                x2sl(b, k, ci * CHUNK, (ci + 1) * CHUNK),
                                start=(k == 0),
                                stop=(k == KT - 1),
                            )
                        # pure copy drain, split ~3:1 ACT:DVE (ACT is
                        # cheaper per chunk and has less other work)
                        if drain_ct[0] % 4 != 3:
                            nc.scalar.copy(stag[:, cs], ps[:])
                        else:
                            nc.vector.tensor_copy(stag[:, cs], ps[:])
                        drain_ct[0] += 1
                    if b == len(BLOCKS) - 1:
                        # finer stores on the last block shorten the tail
                        for ci in range(n_chunks):
                            cs = slice(ci * CHUNK, (ci + 1) * CHUNK)
                            nc.sync.dma_start(
                                out=out_d[
                                    m * P : (m + 1) * P,
                                    off + ci * CHUNK : off + (ci + 1) * CHUNK,
                                ],
                                in_=stag[:, cs],
                            )
                    else:
                        nc.sync.dma_start(
                            out=out_d[m * P : (m + 1) * P, off : off + w],
                            in_=stag[:],
                        )

            x2_chain(0)
            for b in range(len(BLOCKS)):
                mains(b)

    nc.compile()
    return nc


def _get_nc():
    if "nc" not in _CACHE:
        _CACHE["nc"] = _build()
    return _CACHE["nc"]


def _prep_in_maps(input1, input2):
    input1 = np.asarray(input1, dtype=np.float32)
    input2 = np.asarray(input2, dtype=np.float32)
    assert input1.shape == (N, D) and input2.shape == (M, D)
    x2t = np.ascontiguousarray(input2.T).astype(NP_DT)
    in_maps = []
    for c in range(N_CORES):
        sl = input1[c * N_SHARD : (c + 1) * N_SHARD]
        x1t = np.ascontiguousarray(sl.T).astype(NP_DT)
        in_maps.append({"x1t": x1t, "x2t": x2t})
    return in_maps


def _run(input1, input2, trace=False, trace_kwargs=None):
    nc = _get_nc()
    in_maps = _prep_in_maps(input1, input2)
    res = run_bass_kernel_spmd(
        nc, in_maps, list(range(N_CORES)), trace=trace, **(trace_kwargs or {})
    )
    out = np.concatenate(
        [res.results[i]["out"] for i in range(N_CORES)], axis=0
    ).astype(np.float32)
    return out, res


def kernel(input1, input2):
    out, _ = _run(input1, input2, trace=False)
    return out
